# revision 7
# baseline (speedup 1.0000x reference)
"""Sparse BERT self-attention (DeBERTa-style one-pass mask) on 8 Trainium2
NeuronCores. Data-parallel over batch: core b handles batch element b.
Measured: ~142-144 us HW exec per core, absmax rel err ~6e-4 vs fp32 reference.

Design:
  - Host pre-transposes x -> xT [D,S] and W -> W^T in fp16 (fp16 matmuls run
    at the same 1 cyc/row as bf16 on the PE but carry 10 mantissa bits), so
    the device needs zero transposes.
  - Q^T/K^T computed head-transposed [D,S]; V natural [S,D] with a ones
    column per head so the ctx matmul accumulates softmax denominators into
    output column 64 for free.
  - Scores are computed transposed (keys on partitions) only for the 192
    keys each query actually attends to (own 64-signal block + 128 terms);
    exp on ScalarE with the 1/8 scale fused; no max-subtraction needed
    (|scores| <= ~5).
  - Context matmuls emit natural [q, Dh+1] tiles; normalization is one
    reciprocal [128,6] + one broadcast multiply per 6-head group.
  - Head-group pipeline (2 groups of 6 heads): scores+exp of group g+1
    overlap ctx matmuls of group g; outputs DMA out per (s-tile, group).

Shapes (hardcoded per problem spec):
  B=8, S=1408, D=768, H=12, Dh=64, L=64 (signal), CDD=20, T=128 (terms),
  AF = CDD*L = 1280.

Mask structure (training-mode one-pass, attention_mask==1 everywhere):
  - cdd query rows [0,1280): candidate c attends to its own 64 signal keys
    plus the 128 term keys  -> 192 keys per query.
  - term query rows [1280,1408): attend among the 128 term rows, with the
    *query* projection used for both sides (reference quirk).

Math notes (exact reassociations used by the kernel):
  - bk never enters: (Q+bq)·bk is constant over keys -> cancels in softmax.
  - bq IS added to Q (per-partition add in the Q^T layout).
  - bv is added after normalization (sum_k p = 1 -> +bv once).
  - exp without max-subtraction: |scores| <= ~5, safe in fp32 psum.
  - denominator: V tiles carry a ones-column per head; the ctx matmul
    accumulates sum(exp) into output column 64.
"""

import sys

sys.path.insert(0, "/opt/trn_rl_repo")

import numpy as np

import concourse.bass as bass
import concourse.mybir as mybir
import concourse.tile as tile
from concourse.bass_utils import run_bass_kernel_spmd

# ---------------------------------------------------------------- constants
B, S, D = 8, 1408, 768
H, Dh = 12, 64
L, CDD, T = 64, 20, 128
AF = CDD * L  # 1280
NDC = D // 128  # 6 chunks of the contraction/output dim
NST = S // 128  # 11 s-tiles
SCALE = 1.0 / 8.0  # 1/sqrt(Dh)

BF16 = mybir.dt.float16  # fp16: same PE rate as bf16, 8x finer mantissa
F32 = mybir.dt.float32

QK_SCHUNKS = [(0, 512), (512, 1024), (1024, 1408)]  # s-chunks for Q/K proj
TERM_QCHUNKS = [(0, 512), (512, 1024), (1024, 1280)]  # cdd query chunks
V_OCHUNKS = [(0, 512), (512, 768)]  # output-dim chunks for V proj


# --------------------------------------------- walrus sem-wait legalization
def _legalize_waits(nc, max_waits=1):
    """This container's walrus rejects more than one sem wait per
    instruction. Hoist excess waits onto NOPs inserted just before the
    instruction on the same engine (engine streams execute in block order,
    so the conjunction of waits is preserved)."""
    from concourse import mybir

    k = 0
    for fn in nc.m.functions:
        for bb in fn.blocks:
            new_list = []
            changed = False
            for inst in bb.instructions:
                si = inst.sync_info
                waits = list(si.on_wait) if si is not None else []
                if len(waits) > max_waits:
                    changed = True
                    for w in waits[:-max_waits]:
                        nop = mybir.InstNoOp(name=f"waitsplit_{k}", ins=[], outs=[])
                        k += 1
                        nop.engine = inst.engine
                        nop.sync_info = mybir.SyncInfo(on_wait=[w], on_update=[])
                        new_list.append(nop)
                    inst.sync_info = mybir.SyncInfo(
                        on_wait=waits[-max_waits:], on_update=list(si.on_update)
                    )
                new_list.append(inst)
            if changed:
                bb.instructions = new_list


def _patch_tile_teardown():
    """Drop the second all-engine barrier of the kernel-tail teardown. The
    first barrier already guarantees every engine is past its last sem wait
    before the gpsimd sem-clears run; for a single-shot NEFF the clears only
    need to complete before gpsimd's own stream ends."""
    import concourse.tile as tile_mod
    from concourse.vector_clock import ScopedClock

    def _patched(self, tick_clock, wait_clock):
        nc = self.nc
        drain_inst = nc.sync.drain()
        wait_clock.add_sem_waits(
            drain_inst.ins, ScopedClock({None: tick_clock.global_clock})
        )
        nc.all_engine_barrier()
        assert self.sems is not None
        popped = nc._tile_sem_poison_stack.pop()
        assert popped is self._sem_poison
        nc.clear_and_free_semaphores(list(self.sems.allocated().values()))

    tile_mod.TileContext._drain_and_barrier = _patched


_patch_tile_teardown()


# ------------------------------------------------------------ bass program
def _build_program():
    nc = bass.Bass()
    AF_ = mybir.ActivationFunctionType

    xT_d = nc.dram_tensor("xT", [D, S], BF16, kind="ExternalInput")
    wqT_d = nc.dram_tensor("wqT", [D, D], BF16, kind="ExternalInput")
    wkT_d = nc.dram_tensor("wkT", [D, D], BF16, kind="ExternalInput")
    wvT_d = nc.dram_tensor("wvT", [D, D], BF16, kind="ExternalInput")
    bq_d = nc.dram_tensor("bq", [128, NDC], F32, kind="ExternalInput")
    out_d = nc.dram_tensor("out", [S, D], F32, kind="ExternalOutput")

    with tile.TileContext(nc) as tc:
        with (
            tc.tile_pool(name="persist", bufs=1) as pp,
            tc.tile_pool(name="exps", bufs=2) as ep,
            tc.tile_pool(name="misc", bufs=4) as mp,
        ):
            # ---------------- input DMA: two HWDGE queues (SP=sync, ACT=
            # scalar), dispatched in arrival-need order. Q proj needs all of
            # wq+xt (3.3MB ~ 9.3us at the HBM BW cap), so those go first,
            # split across both queues; wk next; wv last.
            bq_all = pp.tile([128, NDC], F32, name="bq_all", tag="bq_all")
            bqt = [bq_all[:, j : j + 1] for j in range(NDC)]
            xt, wt = [], {"q": [], "k": [], "v": []}
            for j in range(NDC):
                t = pp.tile([128, S], BF16, name=f"xt{j}", tag=f"xt{j}")
                xt.append(t)
                for nm in ("q", "k", "v"):
                    w = pp.tile([128, D], BF16, name=f"w{nm}{j}", tag=f"w{nm}{j}")
                    wt[nm].append(w)
            sync_order = [
                (wt["q"][0], wqT_d, 0), (wt["q"][2], wqT_d, 2), (wt["q"][4], wqT_d, 4),
                (xt[1], xT_d, 1), (xt[3], xT_d, 3), (xt[5], xT_d, 5),
                (wt["k"][0], wkT_d, 0), (wt["k"][2], wkT_d, 2), (wt["k"][4], wkT_d, 4),
                (wt["v"][0], wvT_d, 0), (wt["v"][2], wvT_d, 2), (wt["v"][4], wvT_d, 4),
            ]
            scalar_order = [
                (wt["q"][1], wqT_d, 1), (wt["q"][3], wqT_d, 3), (wt["q"][5], wqT_d, 5),
                (xt[0], xT_d, 0), (xt[2], xT_d, 2), (xt[4], xT_d, 4),
                (wt["k"][1], wkT_d, 1), (wt["k"][3], wkT_d, 3), (wt["k"][5], wkT_d, 5),
                (wt["v"][1], wvT_d, 1), (wt["v"][3], wvT_d, 3), (wt["v"][5], wvT_d, 5),
            ]
            nc.scalar.dma_start(out=bq_all, in_=bq_d[:, :])
            for dst, src, j in sync_order:
                nc.sync.dma_start(out=dst, in_=src[j * 128 : (j + 1) * 128, :])
            for dst, src, j in scalar_order:
                nc.scalar.dma_start(out=dst, in_=src[j * 128 : (j + 1) * 128, :])
            QT = [pp.tile([128, S], BF16, name=f"qT{j}", tag=f"qT{j}") for j in range(NDC)]
            KT = [pp.tile([128, S], BF16, name=f"kT{j}", tag=f"kT{j}") for j in range(NDC)]
            # V tiles: [128, H, Dh+1]; column Dh holds ones (denominator).
            V = [pp.tile([128, H, Dh + 1], BF16, name=f"v{st}", tag=f"v{st}") for st in range(NST)]

            # ---------------- projections
            # Single PSUM budget (8 banks): proj 2, st 2, sga 1, small 1, ctx 2.
            with (
                tc.tile_pool(name="pst", bufs=2, space=bass.MemorySpace.PSUM) as pst,
                tc.tile_pool(name="psg", bufs=1, space=bass.MemorySpace.PSUM) as psg,
                tc.tile_pool(name="psm", bufs=1, space=bass.MemorySpace.PSUM) as psm,
            ):
              with tc.tile_pool(name="pproj", bufs=2, space=bass.MemorySpace.PSUM) as pj:
                  # HAM warm-up: the PE clock gate needs ~3.4us of activity to
                  # reach 2.4GHz, and the PE would otherwise idle ~9us
                  # waiting for the Q-projection input set. Chew on a memset
                  # scratch tile so the real projections start warm.
                  wsrc = pp.tile([128, 512], BF16, name="warm_src", tag="warm_src")
                  nc.vector.memset(wsrc, 1.0)
                  wps = pj.tile([128, 512], F32, name="warm_ps", tag="proj")
                  for _ in range(24):
                      nc.tensor.matmul(
                          wps, lhsT=wsrc[:, 0:128], rhs=wsrc, start=True, stop=True
                      )
                  # read once so the psum buf releases back to the pool
                  # (Vector: ScalarE must stay free to dispatch input DMAs)
                  nc.vector.tensor_copy(out=wsrc[:, 0:1], in_=wps[:, 0:1])
                  for oc in range(NDC):
                      for s0, s1 in QK_SCHUNKS:
                          w = s1 - s0
                          pq = pj.tile([128, 512], F32, name="pq", tag="proj")
                          for dc in range(NDC):
                              nc.tensor.matmul(
                                  pq[:, :w],
                                  lhsT=wt["q"][dc][:, oc * 128 : (oc + 1) * 128],
                                  rhs=xt[dc][:, s0:s1],
                                  start=(dc == 0),
                                  stop=(dc == NDC - 1),
                              )
                          # Q^T = psum + bq (per-partition), cast to bf16
                          nc.vector.tensor_scalar_add(
                              out=QT[oc][:, s0:s1], in0=pq[:, :w], scalar1=bqt[oc]
                          )
                  for oc in range(NDC):
                      for s0, s1 in QK_SCHUNKS:
                          w = s1 - s0
                          pk = pj.tile([128, 512], F32, name="pk", tag="proj")
                          for dc in range(NDC):
                              nc.tensor.matmul(
                                  pk[:, :w],
                                  lhsT=wt["k"][dc][:, oc * 128 : (oc + 1) * 128],
                                  rhs=xt[dc][:, s0:s1],
                                  start=(dc == 0),
                                  stop=(dc == NDC - 1),
                              )
                          nc.scalar.activation(
                              out=KT[oc][:, s0:s1], in_=pk[:, :w], func=AF_.Copy
                          )
                  for st in range(NST):
                      for o0, o1 in V_OCHUNKS:
                          w = o1 - o0
                          pv = pj.tile([128, 512], F32, name="pv", tag="proj")
                          for dc in range(NDC):
                              nc.tensor.matmul(
                                  pv[:, :w],
                                  lhsT=xt[dc][:, st * 128 : (st + 1) * 128],
                                  rhs=wt["v"][dc][:, o0:o1],
                                  start=(dc == 0),
                                  stop=(dc == NDC - 1),
                              )
                          nh = w // Dh
                          h0 = o0 // Dh
                          nc.vector.tensor_copy(
                              out=V[st][:, h0 : h0 + nh, 0:Dh],
                              in_=pv[:, :w].rearrange("p (h d) -> p h d", d=Dh),
                          )
                      nc.vector.memset(V[st][:, :, Dh : Dh + 1], 1.0)

              # ------- head-group pipeline: scores+exp for 4 heads, then ctx
              with tc.tile_pool(name="pctx", bufs=3, space=bass.MemorySpace.PSUM) as pctx:
                for hg in range(2):
                    ET, EG, EP = {}, {}, {}
                    for hpair in range(3):
                        h0 = hg * 6 + hpair * 2  # heads h0 (rows 0-63), h0+1
                        j = h0 // 2
                        qa, ka = QT[j][0:Dh, :], KT[j][0:Dh, :]
                        qb, kb = QT[j][Dh:128, :], KT[j][Dh:128, :]

                        # term scores for both heads of the pair
                        for h, qh, kh in ((h0, qa, ka), (h0 + 1, qb, kb)):
                            et = pp.tile([128, AF], BF16, name=f"et{h}", tag=f"et{h}")
                            for s0, s1 in TERM_QCHUNKS:
                                w = s1 - s0
                                stp = pst.tile([128, 512], F32, name="stp", tag="st")
                                nc.tensor.matmul(
                                    stp[:, :w],
                                    lhsT=kh[:, AF:S],
                                    rhs=qh[:, s0:s1],
                                    start=True,
                                    stop=True,
                                )
                                nc.scalar.activation(
                                    out=et[:, s0:s1],
                                    in_=stp[:, :w],
                                    func=AF_.Exp,
                                    scale=SCALE,
                                )
                            ET[h] = et

                        # sig scores: interleave the two heads with opposite
                        # candidate parity -> disjoint (row, col) array
                        # quadrants -> 4-way concurrent matmuls
                        sg = {}
                        for h in (h0, h0 + 1):
                            sg[h] = (
                                psg.tile([128, 512], F32, name=f"sga{h%2}", tag=f"sga{h%2}"),
                                psm.tile([128, 128], F32, name=f"sgb{h%2}", tag="small"),
                            )
                        for c0 in range(CDD):
                            for h, qh, kh, c in (
                                (h0, qa, ka, c0),
                                (h0 + 1, qb, kb, c0 ^ 1),
                            ):
                                row = (c % 2) * Dh
                                sga, sgb = sg[h]
                                if c < 16:
                                    dst = sga[
                                        row : row + Dh,
                                        (c // 2) * 64 : (c // 2) * 64 + 64,
                                    ]
                                else:
                                    cb = (c // 2 - 8) * 64
                                    dst = sgb[row : row + Dh, cb : cb + 64]
                                nc.tensor.matmul(
                                    dst,
                                    lhsT=kh[:, c * L : (c + 1) * L],
                                    rhs=qh[:, c * L : (c + 1) * L],
                                    start=True,
                                    stop=True,
                                )
                        for h, qh, kh in ((h0, qa, ka), (h0 + 1, qb, kb)):
                            sga, sgb = sg[h]
                            eg = pp.tile([128, 640], BF16, name=f"eg{h}", tag=f"eg{h}")
                            nc.scalar.activation(
                                out=eg[:, 0:512], in_=sga, func=AF_.Exp, scale=SCALE
                            )
                            nc.scalar.activation(
                                out=eg[:, 512:640], in_=sgb, func=AF_.Exp, scale=SCALE
                            )
                            spp = psm.tile([128, 128], F32, name="spp", tag="small")
                            nc.tensor.matmul(
                                spp,
                                lhsT=qh[:, AF:S],
                                rhs=qh[:, AF:S],
                                start=True,
                                stop=True,
                            )
                            epp = pp.tile([128, 128], BF16, name=f"ep{h}", tag=f"ep{h}")
                            nc.scalar.activation(
                                out=epp, in_=spp, func=AF_.Exp, scale=SCALE
                            )
                            EG[h], EP[h] = eg, epp

                    for t in range(NST):
                        cps = pctx.tile([128, 6, Dh + 1], F32, name="cps", tag="ctx")
                        # 128-row matmuls back-to-back first (pipeline at
                        # ~54ns), then the 64-row sig pairs. start=True clears
                        # has_written for the WHOLE bank -> first matmul only.
                        for hi in range(6):
                            h = hg * 6 + hi
                            nc.tensor.matmul(
                                cps[:, hi, :],
                                lhsT=ET[h][:, t * 128 : (t + 1) * 128]
                                if t < 10
                                else EP[h],
                                rhs=V[NST - 1][:, h, :],
                                start=(hi == 0),
                                stop=(t == 10 and hi == 5),
                            )
                        if t < 10:
                            for hi in range(6):
                                h = hg * 6 + hi
                                nc.tensor.matmul(
                                    cps[0:64, hi, :],
                                    lhsT=EG[h][0:64, t * 64 : t * 64 + 64],
                                    rhs=V[t][0:64, h, :],
                                    start=False,
                                    stop=(hi == 5),
                                )
                                nc.tensor.matmul(
                                    cps[64:128, hi, :],
                                    lhsT=EG[h][64:128, t * 64 : t * 64 + 64],
                                    rhs=V[t][64:128, h, :],
                                    start=False,
                                    stop=(hi == 5),
                                )
                        rc = mp.tile([128, 6], F32, name="rc", tag="rc")
                        nc.vector.reciprocal(out=rc, in_=cps[:, :, Dh : Dh + 1])
                        ot = mp.tile([128, 6, Dh], F32, name="ot", tag="ot", bufs=6)
                        nc.vector.tensor_mul(
                            out=ot,
                            in0=cps[:, :, 0:Dh],
                            in1=rc.to_broadcast([128, 6, Dh]),
                        )
                        # alternate the two HWDGE queues (SP / ACT) so output
                        # DMA receipt round-trips pipeline 2-wide
                        dma_eng = nc.scalar if (t + hg) % 2 else nc.sync
                        dma_eng.dma_start(
                            out=out_d[
                                t * 128 : (t + 1) * 128, hg * 384 : (hg + 1) * 384
                            ],
                            in_=ot,
                        )

    _legalize_waits(nc)
    return nc


_NC = None


def _get_nc():
    global _NC
    if _NC is None:
        _NC = _build_program()
    return _NC


# -------------------------------------------------------------- host wrapper
def _prep_inputs(hidden_states, Wq, bq, Wk, Wv, bv):
    bf = np.float16
    hs = np.asarray(hidden_states, dtype=np.float32)
    wq = np.asarray(Wq, dtype=np.float32)
    wk = np.asarray(Wk, dtype=np.float32)
    wv = np.asarray(Wv, dtype=np.float32)
    bq = np.asarray(bq, dtype=np.float32)
    bv = np.asarray(bv, dtype=np.float32)

    # W is [out, in]; device wants W^T = [in, out] (contraction on partitions)
    wqT = np.ascontiguousarray(wq.T).astype(bf)
    wkT = np.ascontiguousarray(wk.T).astype(bf)
    wvT = np.ascontiguousarray(wv.T).astype(bf)
    bq6 = np.ascontiguousarray(bq.reshape(NDC, 128).T)

    in_maps = []
    for b in range(B):
        xT = np.ascontiguousarray(hs[b].T).astype(bf)
        in_maps.append(
            {
                "xT": xT,
                "wqT": wqT,
                "wkT": wkT,
                "wvT": wvT,
                "bq": bq6,
            }
        )
    return in_maps


def _enable_tracing():
    """This image lacks ``antenv.axon_hooks``; recreate the NTFF profile hook
    from the boot package's ctypes impl, and defang the artifact upload."""
    import types

    import antenv

    if "antenv.axon_hooks" not in sys.modules:
        from trn_agent_boot.trn_boot import _ntff_profile_via_ctypes

        hook = _ntff_profile_via_ctypes("/opt/axon/libaxon_pjrt.so")
        mod = types.ModuleType("antenv.axon_hooks")
        mod.get_axon_ntff_profile_hook = lambda: hook
        mod.set_axon_ntff_profile_hook = lambda h: None
        sys.modules["antenv.axon_hooks"] = mod
        antenv.axon_hooks = mod
    import concourse.bass_utils as bu

    bu.upload_artifacts = lambda tmpdir: tmpdir


def run(inputs, trace=False, tmpdir=None):
    """Returns (output [B,S,D] f32, BassKernelResults)."""
    if trace:
        _enable_tracing()
    assert int(inputs["num_heads"]) == H
    assert int(inputs["signal_length"]) == L
    assert int(inputs["cdd_size"]) == CDD
    assert int(inputs["term_num"]) == T
    nc = _get_nc()
    in_maps = _prep_inputs(
        inputs["hidden_states"],
        inputs["Wq"],
        inputs["bq"],
        inputs["Wk"],
        inputs["Wv"],
        inputs["bv"],
    )
    res = run_bass_kernel_spmd(
        nc, in_maps, list(range(B)), trace=trace, tmpdir=tmpdir
    )
    out = np.stack([res.results[c]["out"] for c in range(B)]).astype(np.float32)
    out += np.asarray(inputs["bv"], dtype=np.float32)[None, None, :]
    return out, res


def kernel(**inputs) -> np.ndarray:
    out, _ = run(inputs, trace=False)
    return out



# revision 8
# speedup vs baseline: 1.1816x; 1.1816x over previous
"""Sparse BERT self-attention (DeBERTa-style one-pass mask) on 8 Trainium2
NeuronCores. Data-parallel over batch: core b handles batch element b.

v2 schedule (from trace analysis of the 144us baseline):
  - Input DMA balanced across the SP/ACT HWDGE queues in need-order
    (wq+xt first, interleaved, so the Q-projection critical set lands at
    the ~9.3us HBM-BW floor instead of ~18us).
  - Extended PE warmup (24 matmuls) covers the DMA window so the Q
    projection starts warm with no HAM ramp.
  - Projections: all-Q then all-K (K weights arrive later), then V is
    NOT a separate phase: V-projection chunk-chains are interleaved with
    the score matmuls in 6 rounds (one head-pair each). This hides the
    ~31us of ScalarE exp work behind PE work and removes the
    exp->psum backpressure stalls the baseline had.
  - Term scores: the two heads of a pair are emitted adjacently with
    lhsT on partitions 0:64 / 64:128 -> disjoint row groups -> the PE
    runs them concurrently (2x).
  - Sig scores keep the baseline 4-way quadrant packing.
  - ctx phase: per (s-tile, head-group) psum tile; normalization
    alternates Vector tensor_mul / ScalarE per-head Copy-with-scale so
    neither engine gates the PE; output is fp16 (host upcasts + adds bv),
    halving the output DMA bytes.

Shapes (hardcoded per problem spec):
  B=8, S=1408, D=768, H=12, Dh=64, L=64 (signal), CDD=20, T=128 (terms),
  AF = CDD*L = 1280.

Math notes (exact reassociations used by the kernel):
  - bk never enters: (Q+bq)*bk is constant over keys -> cancels in softmax.
  - bq IS added to Q (per-partition add in the Q^T layout).
  - bv is added after normalization (sum_k p = 1 -> +bv once, on host).
  - exp without max-subtraction: |scores| <= ~5, safe in fp32 psum.
  - denominator: V tiles carry a ones-column per head; the ctx matmul
    accumulates sum(exp) into output column 64.
"""

import sys

sys.path.insert(0, "/opt/trn_rl_repo")

import numpy as np

import concourse.bass as bass
import concourse.mybir as mybir
import concourse.tile as tile
from concourse.bass_utils import run_bass_kernel_spmd

# ---------------------------------------------------------------- constants
B, S, D = 8, 1408, 768
H, Dh = 12, 64
L, CDD, T = 64, 20, 128
AF = CDD * L  # 1280
NDC = D // 128  # 6 chunks of the contraction/output dim
NST = S // 128  # 11 s-tiles
SCALE = 1.0 / 8.0  # 1/sqrt(Dh)

BF16 = mybir.dt.float16  # fp16: same PE rate as bf16, 8x finer mantissa
F32 = mybir.dt.float32

QK_SCHUNKS = [(0, 512), (512, 1024), (1024, 1408)]  # s-chunks for Q/K proj
TERM_QCHUNKS = [(0, 512), (512, 1024), (1024, 1280)]  # cdd query chunks
V_OCHUNKS = [(0, 512), (512, 768)]  # output-dim chunks for V proj
SIG_SEGS = [range(0, 7), range(7, 14), range(14, 20)]  # candidate segments
WARMUP_MMS = 24


# --------------------------------------------- walrus sem-wait legalization
def _legalize_waits(nc, max_waits=1):
    """This container's walrus rejects more than one sem wait per
    instruction. Hoist excess waits onto NOPs inserted just before the
    instruction on the same engine (engine streams execute in block order,
    so the conjunction of waits is preserved)."""
    from concourse import mybir

    k = 0
    for fn in nc.m.functions:
        for bb in fn.blocks:
            new_list = []
            changed = False
            for inst in bb.instructions:
                si = inst.sync_info
                waits = list(si.on_wait) if si is not None else []
                if len(waits) > max_waits:
                    changed = True
                    for w in waits[:-max_waits]:
                        nop = mybir.InstNoOp(name=f"waitsplit_{k}", ins=[], outs=[])
                        k += 1
                        nop.engine = inst.engine
                        nop.sync_info = mybir.SyncInfo(on_wait=[w], on_update=[])
                        new_list.append(nop)
                    inst.sync_info = mybir.SyncInfo(
                        on_wait=waits[-max_waits:], on_update=list(si.on_update)
                    )
                new_list.append(inst)
            if changed:
                bb.instructions = new_list


def _patch_tile_teardown():
    """Drop the second all-engine barrier of the kernel-tail teardown."""
    import concourse.tile as tile_mod
    from concourse.vector_clock import ScopedClock

    def _patched(self, tick_clock, wait_clock):
        nc = self.nc
        drain_inst = nc.sync.drain()
        wait_clock.add_sem_waits(
            drain_inst.ins, ScopedClock({None: tick_clock.global_clock})
        )
        nc.all_engine_barrier()
        assert self.sems is not None
        popped = nc._tile_sem_poison_stack.pop()
        assert popped is self._sem_poison
        nc.clear_and_free_semaphores(list(self.sems.allocated().values()))

    tile_mod.TileContext._drain_and_barrier = _patched


_patch_tile_teardown()


# ------------------------------------------------------------ bass program
def _build_program(legalize=True):
    nc = bass.Bass()
    AF_ = mybir.ActivationFunctionType

    xT_d = nc.dram_tensor("xT", [D, S], BF16, kind="ExternalInput")
    wqT_d = nc.dram_tensor("wqT", [D, D], BF16, kind="ExternalInput")
    wkT_d = nc.dram_tensor("wkT", [D, D], BF16, kind="ExternalInput")
    wvT_d = nc.dram_tensor("wvT", [D, D], BF16, kind="ExternalInput")
    bq_d = nc.dram_tensor("bq", [128, NDC], F32, kind="ExternalInput")
    out_d = nc.dram_tensor("out", [S, D], BF16, kind="ExternalOutput")

    with tile.TileContext(nc) as tc:
        with (
            tc.tile_pool(name="persist", bufs=1) as pp,
            tc.tile_pool(name="misc", bufs=4) as mp,
        ):
            # ---------------- input DMA: two HWDGE queues (SP=sync, ACT=
            # scalar), dispatched in arrival-need order. Q proj needs all of
            # wq+xt (3.3MB ~ 9.3us at the HBM BW cap), so those go first,
            # split across both queues; wk next; wv last.
            bq_all = pp.tile([128, NDC], F32, name="bq_all", tag="bq_all")
            xt, wt = [], {"q": [], "k": [], "v": []}
            for j in range(NDC):
                t = pp.tile([128, S], BF16, name=f"xt{j}", tag=f"xt{j}")
                xt.append(t)
                for nm in ("q", "k", "v"):
                    w = pp.tile([128, D], BF16, name=f"w{nm}{j}", tag=f"w{nm}{j}")
                    wt[nm].append(w)
            sync_order = [
                (wt["q"][0], wqT_d, 0), (wt["q"][2], wqT_d, 2), (wt["q"][4], wqT_d, 4),
                (xt[1], xT_d, 1), (xt[3], xT_d, 3), (xt[5], xT_d, 5),
                (wt["k"][0], wkT_d, 0), (wt["k"][2], wkT_d, 2), (wt["k"][4], wkT_d, 4),
                (wt["v"][0], wvT_d, 0), (wt["v"][2], wvT_d, 2), (wt["v"][4], wvT_d, 4),
            ]
            scalar_order = [
                (wt["q"][1], wqT_d, 1), (wt["q"][3], wqT_d, 3), (wt["q"][5], wqT_d, 5),
                (xt[0], xT_d, 0), (xt[2], xT_d, 2), (xt[4], xT_d, 4),
                (wt["k"][1], wkT_d, 1), (wt["k"][3], wkT_d, 3), (wt["k"][5], wkT_d, 5),
                (wt["v"][1], wvT_d, 1), (wt["v"][3], wvT_d, 3), (wt["v"][5], wvT_d, 5),
            ]
            nc.scalar.dma_start(out=bq_all, in_=bq_d[:, :])
            for dst, src, j in sync_order:
                nc.sync.dma_start(out=dst, in_=src[j * 128 : (j + 1) * 128, :])
            for dst, src, j in scalar_order:
                nc.scalar.dma_start(out=dst, in_=src[j * 128 : (j + 1) * 128, :])

            bqt = [bq_all[:, j : j + 1] for j in range(NDC)]
            QT = [pp.tile([128, S], BF16, name=f"qT{j}", tag=f"qT{j}") for j in range(NDC)]
            KT = [pp.tile([128, S], BF16, name=f"kT{j}", tag=f"kT{j}") for j in range(NDC)]
            # V tiles: [128, H, Dh+1]; column Dh holds ones (denominator).
            V = [pp.tile([128, H, Dh + 1], BF16, name=f"v{st}", tag=f"v{st}") for st in range(NST)]

            # ---------------- HAM warm-up covering the input-DMA window.
            wsrc = pp.tile([128, 512], BF16, name="warm_src", tag="warm_src")
            nc.vector.memset(wsrc, 1.0)
            with tc.tile_pool(name="pwarm", bufs=1, space=bass.MemorySpace.PSUM) as pw:
                wps = pw.tile([128, 512], F32, name="warm_ps", tag="warm")
                for _ in range(WARMUP_MMS):
                    nc.tensor.matmul(
                        wps, lhsT=wsrc[:, 0:128], rhs=wsrc, start=True, stop=True
                    )
                # release read (Vector; Scalar must stay free for DMA dispatch)
                nc.vector.tensor_copy(out=wsrc[:, 0:1], in_=wps[:, 0:1])

            # ---------------- Q then K projections (K weights land later)
            with tc.tile_pool(name="pproj", bufs=4, space=bass.MemorySpace.PSUM) as pj:
                for oc in range(NDC):
                    for s0, s1 in QK_SCHUNKS:
                        w = s1 - s0
                        pq = pj.tile([128, 512], F32, name="pq", tag="proj")
                        for dc in range(NDC):
                            nc.tensor.matmul(
                                pq[:, :w],
                                lhsT=wt["q"][dc][:, oc * 128 : (oc + 1) * 128],
                                rhs=xt[dc][:, s0:s1],
                                start=(dc == 0),
                                stop=(dc == NDC - 1),
                            )
                        # Q^T = psum + bq (per-partition), cast to fp16
                        nc.vector.tensor_scalar_add(
                            out=QT[oc][:, s0:s1], in0=pq[:, :w], scalar1=bqt[oc]
                        )
                for oc in range(NDC):
                    for s0, s1 in QK_SCHUNKS:
                        w = s1 - s0
                        pk = pj.tile([128, 512], F32, name="pk", tag="proj")
                        for dc in range(NDC):
                            nc.tensor.matmul(
                                pk[:, :w],
                                lhsT=wt["k"][dc][:, oc * 128 : (oc + 1) * 128],
                                rhs=xt[dc][:, s0:s1],
                                start=(dc == 0),
                                stop=(dc == NDC - 1),
                            )
                        nc.scalar.activation(
                            out=KT[oc][:, s0:s1], in_=pk[:, :w], func=AF_.Copy
                        )

            # ---------------- combined phase: 6 rounds, one head-pair each.
            ET, EG, EP = {}, {}, {}
            v_round = [[10, 0], [1, 2], [3, 4], [5, 6], [7, 8], [9]]
            with (
                tc.tile_pool(name="pst", bufs=1, space=bass.MemorySpace.PSUM) as pst,
                tc.tile_pool(name="psg", bufs=1, space=bass.MemorySpace.PSUM) as psg,
                tc.tile_pool(name="psm", bufs=1, space=bass.MemorySpace.PSUM) as psm,
                tc.tile_pool(name="pv", bufs=2, space=bass.MemorySpace.PSUM) as pv,
            ):
                def v_chain(st, och):
                    o0, o1 = V_OCHUNKS[och]
                    w = o1 - o0
                    pvt = pv.tile([128, 512], F32, name="pv", tag="pv")
                    for dc in range(NDC):
                        nc.tensor.matmul(
                            pvt[:, :w],
                            lhsT=xt[dc][:, st * 128 : (st + 1) * 128],
                            rhs=wt["v"][dc][:, o0:o1],
                            start=(dc == 0),
                            stop=(dc == NDC - 1),
                        )
                    nh = w // Dh
                    h0_ = o0 // Dh
                    nc.vector.tensor_copy(
                        out=V[st][:, h0_ : h0_ + nh, 0:Dh],
                        in_=pvt[:, :w].rearrange("p (h d) -> p h d", d=Dh),
                    )
                    if och == len(V_OCHUNKS) - 1:
                        nc.vector.memset(V[st][:, :, Dh : Dh + 1], 1.0)

                for r in range(6):
                    h0 = 2 * r
                    qa, ka = QT[r][0:Dh, :], KT[r][0:Dh, :]
                    qb, kb = QT[r][Dh:128, :], KT[r][Dh:128, :]
                    et_a = pp.tile([128, AF], BF16, name=f"et{h0}", tag=f"et{h0}")
                    et_b = pp.tile([128, AF], BF16, name=f"et{h0+1}", tag=f"et{h0+1}")
                    ET[h0], ET[h0 + 1] = et_a, et_b
                    sga = {
                        0: psg.tile([128, 512], F32, name="sga0", tag="sga0"),
                        1: psg.tile([128, 512], F32, name="sga1", tag="sga1"),
                    }
                    sgb = {
                        0: psm.tile([128, 128], F32, name="sgb0", tag="small"),
                        1: psm.tile([128, 128], F32, name="sgb1", tag="small"),
                    }

                    vslots = []
                    for st in v_round[r]:
                        vslots.append((st, 0))
                        vslots.append((st, 1))

                    for ci, (s0, s1) in enumerate(TERM_QCHUNKS):
                        w = s1 - s0
                        # term pair: head A on array rows 0:63, head B on
                        # 64:127 -> concurrent matmuls
                        pa = pst.tile([128, 512], F32, name="pa", tag="ta", bufs=2)
                        nc.tensor.matmul(
                            pa[:, :w], lhsT=ka[:, AF:S], rhs=qa[:, s0:s1],
                            start=True, stop=True,
                        )
                        pb = pst.tile([128, 512], F32, name="pb", tag="tb", bufs=1)
                        nc.tensor.matmul(
                            pb[:, :w], lhsT=kb[:, AF:S], rhs=qb[:, s0:s1],
                            start=True, stop=True,
                        )
                        nc.scalar.activation(
                            out=et_a[:, s0:s1], in_=pa[:, :w], func=AF_.Exp, scale=SCALE
                        )
                        nc.scalar.activation(
                            out=et_b[:, s0:s1], in_=pb[:, :w], func=AF_.Exp, scale=SCALE
                        )
                        # sig segment: interleave the two heads with opposite
                        # candidate parity -> disjoint (row, col) array
                        # quadrants -> 4-way concurrent matmuls
                        for c0 in SIG_SEGS[ci]:
                            for h, qh, kh, c in (
                                (h0, qa, ka, c0),
                                (h0 + 1, qb, kb, c0 ^ 1),
                            ):
                                row = (c % 2) * Dh
                                if c < 16:
                                    dst = sga[h % 2][
                                        row : row + Dh,
                                        (c // 2) * 64 : (c // 2) * 64 + 64,
                                    ]
                                else:
                                    cb = (c // 2 - 8) * 64
                                    dst = sgb[h % 2][row : row + Dh, cb : cb + 64]
                                nc.tensor.matmul(
                                    dst,
                                    lhsT=kh[:, c * L : (c + 1) * L],
                                    rhs=qh[:, c * L : (c + 1) * L],
                                    start=True,
                                    stop=True,
                                )
                        # V-projection chunk chain keeps the PE busy while
                        # ScalarE drains the term-score psums
                        if ci < len(vslots):
                            v_chain(*vslots[ci])

                    for h, qh in ((h0, qa), (h0 + 1, qb)):
                        eg = pp.tile([128, 640], BF16, name=f"eg{h}", tag=f"eg{h}")
                        nc.scalar.activation(
                            out=eg[:, 0:512], in_=sga[h % 2], func=AF_.Exp, scale=SCALE
                        )
                        nc.scalar.activation(
                            out=eg[:, 512:640], in_=sgb[h % 2], func=AF_.Exp,
                            scale=SCALE,
                        )
                        spp = psm.tile([128, 128], F32, name="spp", tag="small")
                        nc.tensor.matmul(
                            spp, lhsT=qh[:, AF:S], rhs=qh[:, AF:S],
                            start=True, stop=True,
                        )
                        epp = pp.tile([128, 128], BF16, name=f"ep{h}", tag=f"ep{h}")
                        nc.scalar.activation(
                            out=epp, in_=spp, func=AF_.Exp, scale=SCALE
                        )
                        EG[h], EP[h] = eg, epp
                    for slot in vslots[len(TERM_QCHUNKS):]:
                        v_chain(*slot)

            # ---------------- ctx phase: st-major over 22 (s-tile, head-group)
            # groups. Normalization alternates Vector tensor_mul / ScalarE
            # per-head Copy-with-scale (ScalarE is idle here) so neither
            # engine gates the PE; fp16 output halves the DMA-out bytes; all
            # output DMAs dispatch on the otherwise-idle SP queue.
            with tc.tile_pool(name="pctx", bufs=4, space=bass.MemorySpace.PSUM) as pctx:
                gi = 0
                for t in range(NST):
                    for hg in range(2):
                        cps = pctx.tile([128, 6, Dh + 1], F32, name="cps", tag="ctx")
                        for hi in range(6):
                            h = hg * 6 + hi
                            nc.tensor.matmul(
                                cps[:, hi, :],
                                lhsT=ET[h][:, t * 128 : (t + 1) * 128]
                                if t < 10
                                else EP[h],
                                rhs=V[NST - 1][:, h, :],
                                start=(hi == 0),
                                stop=(t == 10 and hi == 5),
                            )
                        if t < 10:
                            for hi in range(6):
                                h = hg * 6 + hi
                                nc.tensor.matmul(
                                    cps[0:64, hi, :],
                                    lhsT=EG[h][0:64, t * 64 : t * 64 + 64],
                                    rhs=V[t][0:64, h, :],
                                    start=False,
                                    stop=(hi == 5),
                                )
                                nc.tensor.matmul(
                                    cps[64:128, hi, :],
                                    lhsT=EG[h][64:128, t * 64 : t * 64 + 64],
                                    rhs=V[t][64:128, h, :],
                                    start=False,
                                    stop=(hi == 5),
                                )
                        rc = mp.tile([128, 6], F32, name="rc", tag="rc")
                        nc.vector.reciprocal(out=rc, in_=cps[:, :, Dh : Dh + 1])
                        ot = mp.tile([128, 6, Dh], BF16, name="ot", tag="ot", bufs=8)
                        if gi % 2 == 0:
                            nc.vector.tensor_mul(
                                out=ot,
                                in0=cps[:, :, 0:Dh],
                                in1=rc.to_broadcast([128, 6, Dh]),
                            )
                        else:
                            for hi in range(6):
                                nc.scalar.activation(
                                    out=ot[:, hi, :],
                                    in_=cps[:, hi, 0:Dh],
                                    func=AF_.Copy,
                                    scale=rc[:, hi : hi + 1],
                                )
                        nc.sync.dma_start(
                            out=out_d[
                                t * 128 : (t + 1) * 128, hg * 384 : (hg + 1) * 384
                            ],
                            in_=ot,
                        )
                        gi += 1

    if legalize:
        _legalize_waits(nc)
    return nc


_NC = None


def _get_nc():
    global _NC
    if _NC is None:
        _NC = _build_program()
    return _NC


# -------------------------------------------------------------- host wrapper
def _prep_inputs(hidden_states, Wq, bq, Wk, Wv):
    bf = np.float16
    hs = np.asarray(hidden_states, dtype=np.float32)
    wq = np.asarray(Wq, dtype=np.float32)
    wk = np.asarray(Wk, dtype=np.float32)
    wv = np.asarray(Wv, dtype=np.float32)
    bq = np.asarray(bq, dtype=np.float32)

    wqT = np.ascontiguousarray(wq.T).astype(bf)
    wkT = np.ascontiguousarray(wk.T).astype(bf)
    wvT = np.ascontiguousarray(wv.T).astype(bf)
    bq6 = np.ascontiguousarray(bq.reshape(NDC, 128).T)

    in_maps = []
    for b in range(B):
        xT = np.ascontiguousarray(hs[b].T).astype(bf)
        in_maps.append(
            {
                "xT": xT,
                "wqT": wqT,
                "wkT": wkT,
                "wvT": wvT,
                "bq": bq6,
            }
        )
    return in_maps


def _enable_tracing():
    """This image lacks ``antenv.axon_hooks``; recreate the NTFF profile hook
    from the boot package's ctypes impl, and defang the artifact upload."""
    import types

    import antenv

    if "antenv.axon_hooks" not in sys.modules:
        from trn_agent_boot.trn_boot import _ntff_profile_via_ctypes

        hook = _ntff_profile_via_ctypes("/opt/axon/libaxon_pjrt.so")
        mod = types.ModuleType("antenv.axon_hooks")
        mod.get_axon_ntff_profile_hook = lambda: hook
        mod.set_axon_ntff_profile_hook = lambda h: None
        sys.modules["antenv.axon_hooks"] = mod
        antenv.axon_hooks = mod
    import concourse.bass_utils as bu

    bu.upload_artifacts = lambda tmpdir: tmpdir


def run(inputs, trace=False, tmpdir=None):
    """Returns (output [B,S,D] f32, BassKernelResults)."""
    if trace:
        _enable_tracing()
    assert int(inputs["num_heads"]) == H
    assert int(inputs["signal_length"]) == L
    assert int(inputs["cdd_size"]) == CDD
    assert int(inputs["term_num"]) == T
    nc = _get_nc()
    in_maps = _prep_inputs(
        inputs["hidden_states"],
        inputs["Wq"],
        inputs["bq"],
        inputs["Wk"],
        inputs["Wv"],
    )
    res = run_bass_kernel_spmd(
        nc, in_maps, list(range(B)), trace=trace, tmpdir=tmpdir
    )
    out = np.stack([res.results[c]["out"] for c in range(B)]).astype(np.float32)
    out += np.asarray(inputs["bv"], dtype=np.float32)[None, None, :]
    return out, res


def kernel(**inputs) -> np.ndarray:
    out, _ = run(inputs, trace=False)
    return out
